# revision 9
# baseline (speedup 1.0000x reference)
"""HMM forward-algorithm loss on 8 NeuronCores (Bass/Tile), two launches.

Math: loss = -mean_n log sum_k alpha_T[n,k] for the linear-domain forward
recursion q_t = (P^T q_{t-1}) . e_{x_t}, P = softmax(rows of trans).

Launch A (V-sharded, 8 cores): emission table shard exp(emb @ voc^T - C0) in
bf16 + f32 partial row sums (for the log-softmax normalizer). All matmul/exp
FLOPs on device; host only reassembles shards.

Host middle step (pure data movement): normalize table columns by the summed
normalizer, gather per-token emission columns and build per-core SIMD lane
streams.

Launch B (batch+chunk-parallel scan): T=4096 is split into C=128 chunks of
L=32 steps; each (sequence, chunk) pair is a SIMD lane (F=512 lanes/core,
4 seqs per core). Each lane runs S = W+L = 40 steps: W=8 warmup steps
re-derive the incoming alpha direction from the preceding real tokens (HMM
forward mixing is exponentially fast; validated rel err ~7e-8), then L real
steps. Lanes are split into two independent chains so PE matmuls of one chain
overlap DVE multiplies of the other. Renorm every R=8 steps extracts log
column sums into an accumulator; snapshots at the chunk boundary let the host
discard warmup mass exactly. Chunk 0 is exact: ones-warmup followed by a
host-built division column that sets q = p0.

Host stitches: contrib = acc_end - snapB + ln(sum q_end) (+ boundary mass for
chunk 0), loss_n = -(sum_c contrib - T ln kappa), kappa a centering constant.
"""

import numpy as np
import ml_dtypes

N, T, K, V = 32, 4096, 128, 50000
P = 128
C0 = 40.0

# launch A: vocab sharding
VPAD = 50176               # 8 * 6272
VSH = VPAD // 8            # vocab rows per core
ACH = 512                  # v-chunk width
NCHA = (VSH + ACH - 1) // ACH   # 13 chunks (12x512 + 1x128)

# launch B: scan layout
C = 128                    # chunks per sequence
L = T // C                 # 32 real steps per chunk
W = 8                      # warmup steps
S = W + L                  # 40 steps per lane
R = 8                      # renorm period (boundary at W is a renorm step)
NSEQ = 4                   # sequences per core
F = NSEQ * C               # 512 lanes per core
H = 2                      # independent chains
FH = F // H                # 256 lanes per chain
G = 8                      # steps per E DMA chunk
NCHB = S // G              # 5 chunks

_CACHE = {}


def _build_nc_a():
    import concourse.mybir as mybir
    import concourse.tile as tile
    from concourse import bacc

    f32 = mybir.dt.float32
    bf16 = mybir.dt.bfloat16
    EXP = mybir.ActivationFunctionType.Exp
    AX = mybir.AxisListType.X

    nc = bacc.Bacc("TRN2", target_bir_lowering=False, debug=False, num_devices=8)

    vocT_d = nc.dram_tensor("vocT", [P, VSH], bf16, kind="ExternalInput")
    embT_d = nc.dram_tensor("embT", [P, P], bf16, kind="ExternalInput")
    tab_d = nc.dram_tensor("tab", [P, VSH], bf16, kind="ExternalOutput")
    sums_d = nc.dram_tensor("sums", [P, 1], f32, kind="ExternalOutput")

    with tile.TileContext(nc) as tc:
        with (
            tc.tile_pool(name="csb", bufs=1) as csb,
            tc.tile_pool(name="sb", bufs=3) as sb,
            tc.tile_pool(name="ps", bufs=2, space="PSUM") as pp,
        ):
            embT = csb.tile([P, P], dtype=bf16)
            nc.sync.dma_start(out=embT[:], in_=embT_d[:, :])
            parts = csb.tile([P, NCHA], dtype=f32)
            negc0 = csb.tile([P, 1], dtype=f32)
            nc.vector.memset(negc0[:], -C0)

            for j in range(NCHA):
                v0 = j * ACH
                vn = min(ACH, VSH - v0)
                vt = sb.tile([P, ACH], dtype=bf16, tag="vt")
                nc.sync.dma_start(out=vt[:, :vn], in_=vocT_d[:, v0 : v0 + vn])
                ps = pp.tile([P, ACH], dtype=f32, tag="l")
                nc.tensor.matmul(
                    out=ps[:, :vn], lhsT=embT[:], rhs=vt[:, :vn], start=True, stop=True
                )
                tb = sb.tile([P, ACH], dtype=bf16, tag="tb")
                nc.scalar.activation(
                    out=tb[:, :vn], in_=ps[:, :vn], func=EXP, bias=negc0[:, :1],
                    accum_out=parts[:, j : j + 1],
                )
                nc.sync.dma_start(out=tab_d[:, v0 : v0 + vn], in_=tb[:, :vn])

            sumt = csb.tile([P, 1], dtype=f32)
            nc.vector.tensor_reduce(
                out=sumt[:], in_=parts[:], axis=AX, op=mybir.AluOpType.add
            )
            nc.sync.dma_start(out=sums_d[:, :], in_=sumt[:])

    if not nc.is_finalized():
        nc.finalize()
    return nc


def _build_nc_b():
    import concourse.mybir as mybir
    import concourse.tile as tile
    from concourse import bacc

    f32 = mybir.dt.float32
    bf16 = mybir.dt.bfloat16
    EXP = mybir.ActivationFunctionType.Exp
    LN = mybir.ActivationFunctionType.Ln
    AX = mybir.AxisListType.X

    nc = bacc.Bacc("TRN2", target_bir_lowering=False, debug=False, num_devices=8)

    e_d = nc.dram_tensor("e", [P, S * F], bf16, kind="ExternalInput")
    tr_d = nc.dram_tensor("tr", [K, K], f32, kind="ExternalInput")
    qf_d = nc.dram_tensor("qf", [P, F], bf16, kind="ExternalOutput")
    stats_d = nc.dram_tensor("stats", [3, F], f32, kind="ExternalOutput")

    renorms = set(range(R, S, R))  # 8,16,24,32

    with tile.TileContext(nc) as tc:
        with (
            tc.tile_pool(name="csb", bufs=1) as csb,
            tc.tile_pool(name="es", bufs=NCHB) as es,
            tc.tile_pool(name="qs", bufs=3) as qs,
            tc.tile_pool(name="rs", bufs=2) as rs,
            tc.tile_pool(name="pmm", bufs=2, space="PSUM") as pmm,
            tc.tile_pool(name="pcs", bufs=2, space="PSUM") as pcs,
            tc.tile_pool(name="pbc", bufs=2, space="PSUM") as pbc,
        ):
            # transition softmax -> P (k par, k' free) bf16, used as lhsT
            trt = csb.tile([P, P], dtype=f32)
            nc.sync.dma_start(out=trt[:], in_=tr_d[:, :])
            rm = csb.tile([P, 1], dtype=f32)
            nc.vector.tensor_reduce(out=rm[:], in_=trt[:], axis=AX, op=mybir.AluOpType.max)
            nrm = csb.tile([P, 1], dtype=f32)
            nc.vector.tensor_scalar_mul(out=nrm[:], in0=rm[:], scalar1=-1.0)
            rsum = csb.tile([P, 1], dtype=f32)
            eL = csb.tile([P, P], dtype=f32)
            nc.scalar.activation(
                out=eL[:], in_=trt[:], func=EXP, bias=nrm[:, :1], accum_out=rsum[:, :1]
            )
            rrs = csb.tile([P, 1], dtype=f32)
            nc.vector.reciprocal(out=rrs[:], in_=rsum[:])
            Pf = csb.tile([P, P], dtype=f32)
            nc.vector.tensor_scalar_mul(out=Pf[:], in0=eL[:], scalar1=rrs[:, :1])
            Pb = csb.tile([P, P], dtype=bf16)
            nc.vector.tensor_copy(out=Pb[:], in_=Pf[:])

            ones_col = csb.tile([P, 1], dtype=bf16)
            nc.vector.memset(ones_col[:], 1.0)
            ones_row = csb.tile([1, P], dtype=bf16)
            nc.vector.memset(ones_row[:], 1.0)

            # emission stream chunks (all live; DMA runs ahead of the scan)
            et = []
            for ch in range(NCHB):
                t_ = es.tile([P, G * F], dtype=bf16, tag="e")
                nc.sync.dma_start(out=t_[:], in_=e_d[:, ch * G * F : (ch + 1) * G * F])
                et.append(t_)

            q = []
            acc, snapA, snapB = [], [], []
            for h in range(H):
                q0 = csb.tile([P, FH], dtype=bf16, tag=f"q0_{h}")
                nc.vector.memset(q0[:], 1.0)
                q.append(q0)
                a_ = csb.tile([1, FH], dtype=f32, tag=f"acc{h}")
                nc.vector.memset(a_[:], 0.0)
                acc.append(a_)
                snapA.append(csb.tile([1, FH], dtype=f32, name=f"snA{h}", tag=f"snA{h}"))
                snapB.append(csb.tile([1, FH], dtype=f32, name=f"snB{h}", tag=f"snB{h}"))

            for step in range(1, S + 1):
                ch, off = divmod(step - 1, G)
                for h in range(H):
                    ps = pmm.tile([P, FH], dtype=f32, tag=f"mm{h}")
                    nc.tensor.matmul(
                        out=ps[:], lhsT=Pb[:], rhs=q[h][:], start=True, stop=True
                    )
                    qn = qs.tile([P, FH], dtype=bf16, tag=f"q{h}")
                    c0 = off * F + h * FH
                    nc.vector.tensor_mul(out=qn[:], in0=ps[:], in1=et[ch][:, c0 : c0 + FH])
                    q[h] = qn
                if step in renorms:
                    for h in range(H):
                        cs = pcs.tile([1, FH], dtype=f32, tag="cs")
                        nc.tensor.matmul(
                            out=cs[:], lhsT=ones_col[:, :1], rhs=q[h][:],
                            start=True, stop=True,
                        )
                        lcs = rs.tile([1, FH], dtype=f32, tag="lcs")
                        nc.scalar.activation(out=lcs[:], in_=cs[:], func=LN)
                        if step == W:
                            nc.vector.tensor_copy(out=snapA[h][:], in_=acc[h][:])
                        nc.vector.tensor_add(out=acc[h][:], in0=acc[h][:], in1=lcs[:])
                        if step == W:
                            nc.vector.tensor_copy(out=snapB[h][:], in_=acc[h][:])
                        rcs = rs.tile([1, FH], dtype=bf16, tag="rcs")
                        with nc.allow_low_precision(
                            reason="renorm scale; its rounding is tracked exactly by acc"
                        ):
                            nc.vector.reciprocal(out=rcs[:], in_=cs[:])
                        bc = pbc.tile([P, FH], dtype=f32, tag="bc")
                        nc.tensor.matmul(
                            out=bc[:], lhsT=ones_row[:1, :], rhs=rcs[:1, :],
                            start=True, stop=True,
                        )
                        qn2 = qs.tile([P, FH], dtype=bf16, tag=f"q{h}")
                        nc.vector.tensor_mul(out=qn2[:], in0=q[h][:], in1=bc[:])
                        q[h] = qn2

            for h in range(H):
                nc.sync.dma_start(out=qf_d[:, h * FH : (h + 1) * FH], in_=q[h][:])
                nc.sync.dma_start(out=stats_d[0:1, h * FH : (h + 1) * FH], in_=snapA[h][:1, :])
                nc.sync.dma_start(out=stats_d[1:2, h * FH : (h + 1) * FH], in_=snapB[h][:1, :])
                nc.sync.dma_start(out=stats_d[2:3, h * FH : (h + 1) * FH], in_=acc[h][:1, :])

    if not nc.is_finalized():
        nc.finalize()
    return nc


def _get_nc(which):
    if which not in _CACHE:
        _CACHE[which] = _build_nc_a() if which == "a" else _build_nc_b()
    return _CACHE[which]


def _run(x, start_w, start_b, cluster_trans_w, emb_cluster_w, cluster_vocab_w,
         trace=False):
    from concourse.bass_utils import run_bass_kernel_spmd

    x = np.asarray(x).astype(np.int64)
    sw = np.asarray(start_w, np.float32).reshape(K)
    sb = np.asarray(start_b, np.float32).reshape(K)
    tr = np.ascontiguousarray(
        np.asarray(cluster_trans_w, np.float32)[:, 0].reshape(K, K)
    )
    emb = np.asarray(emb_cluster_w, np.float32)
    voc = np.asarray(cluster_vocab_w, np.float32)

    # ---------------- launch A: emission table shards ----------------
    vocT = np.zeros((P, VPAD), ml_dtypes.bfloat16)
    vocT[:, :V] = voc.T.astype(ml_dtypes.bfloat16)
    embT = np.ascontiguousarray(emb.T).astype(ml_dtypes.bfloat16)
    nca = _get_nc("a")
    in_a = [
        {"vocT": np.ascontiguousarray(vocT[:, c * VSH : (c + 1) * VSH]), "embT": embT}
        for c in range(8)
    ]
    ra = run_bass_kernel_spmd(nca, in_a, list(range(8)), trace=trace)
    exec_a = ra.exec_time_ns
    res_a = ra.results

    TAB = np.concatenate(
        [res_a[c]["tab"].astype(np.float32) for c in range(8)], axis=1
    )  # (K, VPAD)
    s = np.sum([res_a[c]["sums"][:, 0].astype(np.float64) for c in range(8)], axis=0)

    # ---------------- host: normalize, gather, build lane streams ----------------
    En = TAB[:, :V] / s[:, None].astype(np.float32)        # (K, V) true p(v|k)
    Gall = En[:, x.reshape(-1)]                            # (K, N*T)
    lnkap = -float(np.mean(np.log(Gall.mean(axis=0, dtype=np.float64))))
    Gall *= np.float32(np.exp(lnkap))                      # centered emissions
    Gall = Gall.reshape(K, N, T)

    # transition (f64) for the exact chunk-0 warmup column
    trd = tr.astype(np.float64)
    Pd = np.exp(trd - trd.max(1, keepdims=True))
    Pd /= Pd.sum(1, keepdims=True)
    A = Pd.T
    w = np.ones(K)
    for st_ in range(1, W):
        w = A @ w
        if st_ % R == 0:
            w = w / w.sum()
    p0 = np.exp((sw + sb).astype(np.float64))
    p0col = (p0 / (A @ w)).astype(np.float32)

    # per-(step, chunk) real-token index; -1 = ones, -2 = p0 column
    tmap = np.empty((S, C), np.int64)
    for si in range(S):
        step = si + 1
        for c in range(C):
            if c == 0:
                tmap[si, 0] = -1 if step < W else (-2 if step == W else step - W - 1)
            else:
                tmap[si, c] = c * L - W + step - 1
    ones_mask = tmap == -1
    p0_mask = tmap == -2
    tclip = np.clip(tmap, 0, T - 1)

    e_maps = []
    for cc in range(8):
        st = np.empty((S, NSEQ, C, K), np.float32)
        for nl in range(NSEQ):
            n = cc * NSEQ + nl
            v = st[:, nl]                                   # view (S,C,K)
            v[...] = Gall[:, n, tclip].transpose(1, 2, 0)
            v[ones_mask] = 1.0
            v[p0_mask] = p0col
        # column index = si*F + nl*C + c  -> (S, NSEQ, C) ordering is exactly that
        e_maps.append(
            {
                "e": np.ascontiguousarray(
                    st.reshape(S * F, K).T.astype(ml_dtypes.bfloat16)
                ),
                "tr": tr,
            }
        )

    # ---------------- launch B: chunked scan ----------------
    ncb = _get_nc("b")
    rb = run_bass_kernel_spmd(ncb, e_maps, list(range(8)), trace=trace)
    exec_b = rb.exec_time_ns
    res_b = rb.results

    # ---------------- host: stitch ----------------
    losses = np.empty(N, np.float64)
    for cc in range(8):
        qf = res_b[cc]["qf"].astype(np.float64)            # (K, F)
        stats = res_b[cc]["stats"].astype(np.float64)      # (3, F)
        snapA_, snapB_, accE = stats[0], stats[1], stats[2]
        contrib = accE - snapB_ + np.log(qf.sum(axis=0))   # (F,)
        contrib = contrib.reshape(NSEQ, C)
        contrib[:, 0] += (snapB_ - snapA_).reshape(NSEQ, C)[:, 0]
        for nl in range(NSEQ):
            n = cc * NSEQ + nl
            losses[n] = -(contrib[nl].sum() - T * lnkap)
    return np.float32(losses.mean()), (exec_a, exec_b)


def kernel(x, start_w, start_b, cluster_trans_w, emb_cluster_w, cluster_vocab_w):
    loss, _ = _run(x, start_w, start_b, cluster_trans_w, emb_cluster_w,
                   cluster_vocab_w)
    return loss


# revision 15
# speedup vs baseline: 1.2732x; 1.2732x over previous
"""HMM forward-algorithm loss on 8 NeuronCores (Bass/Tile), two launches.

Math: loss = -mean_n log sum_k alpha_T[n,k] for the linear-domain forward
recursion q_t = (P^T q_{t-1}) . e_{x_t}, P = softmax(rows of trans).

Launch A (V-sharded, 8 cores): emission table shard exp(emb @ voc^T - C0) in
bf16 + f32 partial row sums (for the log-softmax normalizer). All matmul/exp
FLOPs on device; host only reassembles shards.

Host middle step (pure data movement): normalize table columns by the summed
normalizer, gather per-token emission columns and build per-core SIMD lane
streams.

Launch B (batch+chunk-parallel scan): T=4096 is split into C=128 chunks of
L=32 steps; each (sequence, chunk) pair is a SIMD lane (F=512 lanes/core,
4 seqs per core). Each lane runs S = W+L = 40 steps: W=8 warmup steps
re-derive the incoming alpha direction from the preceding real tokens (HMM
forward mixing is exponentially fast; validated rel err ~7e-8), then L real
steps. Lanes are split into two independent chains so PE matmuls of one chain
overlap DVE multiplies of the other. Renorm every R=8 steps extracts log
column sums into an accumulator; snapshots at the chunk boundary let the host
discard warmup mass exactly. Chunk 0 is exact: ones-warmup followed by a
host-built division column that sets q = p0.

Host stitches: contrib = acc_end - snapB + ln(sum q_end) (+ boundary mass for
chunk 0), loss_n = -(sum_c contrib - T ln kappa), kappa a centering constant.
"""

import numpy as np
import ml_dtypes

N, T, K, V = 32, 4096, 128, 50000
P = 128
C0 = 40.0

# launch A: vocab sharding
VPAD = 50176               # 8 * 6272
VSH = VPAD // 8            # vocab rows per core
ACH = 2048                 # v-chunk width (4 matmuls of 512 each)
NCHA = (VSH + ACH - 1) // ACH   # 4 chunks (3x2048 + 128)
MMW = 512                  # matmul moving width

# launch B: scan layout
C = 128                    # chunks per sequence
L = T // C                 # 32 real steps per chunk
W = 4                      # warmup steps
S = W + L                  # 36 steps per lane
RENS = (4, 16, 28)         # renorm steps (first is the chunk boundary)
NSEQ = 4                   # sequences per core
F = NSEQ * C               # 512 lanes per core
H = 2                      # independent chains
FH = F // H                # 256 lanes per chain
G = 6                      # steps per E DMA chunk
NCHB = S // G              # 6 chunks

_CACHE = {}


def _build_nc_a():
    import concourse.mybir as mybir
    import concourse.tile as tile
    from concourse import bacc

    f32 = mybir.dt.float32
    bf16 = mybir.dt.bfloat16
    EXP = mybir.ActivationFunctionType.Exp
    AX = mybir.AxisListType.X

    nc = bacc.Bacc("TRN2", target_bir_lowering=False, debug=False, num_devices=8)

    vocT_d = nc.dram_tensor("vocT", [P, VSH], bf16, kind="ExternalInput")
    embT_d = nc.dram_tensor("embT", [P, P], bf16, kind="ExternalInput")
    tab_d = nc.dram_tensor("tab", [P, VSH], bf16, kind="ExternalOutput")
    sums_d = nc.dram_tensor("sums", [P, 1], f32, kind="ExternalOutput")

    with tile.TileContext(nc) as tc:
        with (
            tc.tile_pool(name="csb", bufs=1) as csb,
            tc.tile_pool(name="sb", bufs=3) as sb,
            tc.tile_pool(name="ps", bufs=2, space="PSUM") as pp,
        ):
            embT = csb.tile([P, P], dtype=bf16)
            nc.sync.dma_start(out=embT[:], in_=embT_d[:, :])
            parts = csb.tile([P, NCHA], dtype=f32)
            negc0 = csb.tile([P, 1], dtype=f32)
            nc.vector.memset(negc0[:], -C0)

            for j in range(NCHA):
                v0 = j * ACH
                vn = min(ACH, VSH - v0)
                vt = sb.tile([P, ACH], dtype=bf16, tag="vt")
                nc.sync.dma_start(out=vt[:, :vn], in_=vocT_d[:, v0 : v0 + vn])
                ps = pp.tile([P, ACH], dtype=f32, tag="l")
                for m0 in range(0, vn, MMW):
                    mn = min(MMW, vn - m0)
                    nc.tensor.matmul(
                        out=ps[:, m0 : m0 + mn], lhsT=embT[:],
                        rhs=vt[:, m0 : m0 + mn], start=True, stop=True,
                    )
                tb = sb.tile([P, ACH], dtype=bf16, tag="tb")
                nc.scalar.activation(
                    out=tb[:, :vn], in_=ps[:, :vn], func=EXP, bias=negc0[:, :1],
                    accum_out=parts[:, j : j + 1],
                )
                nc.sync.dma_start(out=tab_d[:, v0 : v0 + vn], in_=tb[:, :vn])

            sumt = csb.tile([P, 1], dtype=f32)
            nc.vector.tensor_reduce(
                out=sumt[:], in_=parts[:], axis=AX, op=mybir.AluOpType.add
            )
            nc.sync.dma_start(out=sums_d[:, :], in_=sumt[:])

    if not nc.is_finalized():
        nc.finalize()
    return nc


def _build_nc_b():
    import concourse.mybir as mybir
    import concourse.tile as tile
    from concourse import bacc

    f32 = mybir.dt.float32
    bf16 = mybir.dt.bfloat16
    EXP = mybir.ActivationFunctionType.Exp
    LN = mybir.ActivationFunctionType.Ln
    AX = mybir.AxisListType.X

    nc = bacc.Bacc("TRN2", target_bir_lowering=False, debug=False, num_devices=8)

    e_d = nc.dram_tensor("e", [P, S * F], bf16, kind="ExternalInput")
    tr_d = nc.dram_tensor("tr", [K, K], f32, kind="ExternalInput")
    qf_d = nc.dram_tensor("qf", [P, F], bf16, kind="ExternalOutput")
    lcs_d = nc.dram_tensor("lcs", [len(RENS), F], f32, kind="ExternalOutput")

    renorms = {st: i for i, st in enumerate(RENS)}

    with tile.TileContext(nc) as tc:
        with (
            tc.tile_pool(name="csb", bufs=1) as csb,
            tc.tile_pool(name="es", bufs=NCHB) as es,
            tc.tile_pool(name="qs", bufs=3) as qs,
            tc.tile_pool(name="rs", bufs=2) as rs,
            tc.tile_pool(name="pmm", bufs=2, space="PSUM") as pmm,
            tc.tile_pool(name="pcs", bufs=2, space="PSUM") as pcs,
            tc.tile_pool(name="pbc", bufs=2, space="PSUM") as pbc,
        ):
            # transition softmax -> P (k par, k' free) bf16, used as lhsT
            trt = csb.tile([P, P], dtype=f32)
            nc.sync.dma_start(out=trt[:], in_=tr_d[:, :])
            rm = csb.tile([P, 1], dtype=f32)
            nc.vector.tensor_reduce(out=rm[:], in_=trt[:], axis=AX, op=mybir.AluOpType.max)
            nrm = csb.tile([P, 1], dtype=f32)
            nc.vector.tensor_scalar_mul(out=nrm[:], in0=rm[:], scalar1=-1.0)
            rsum = csb.tile([P, 1], dtype=f32)
            eL = csb.tile([P, P], dtype=f32)
            nc.scalar.activation(
                out=eL[:], in_=trt[:], func=EXP, bias=nrm[:, :1], accum_out=rsum[:, :1]
            )
            rrs = csb.tile([P, 1], dtype=f32)
            nc.vector.reciprocal(out=rrs[:], in_=rsum[:])
            Pf = csb.tile([P, P], dtype=f32)
            nc.vector.tensor_scalar_mul(out=Pf[:], in0=eL[:], scalar1=rrs[:, :1])
            Pb = csb.tile([P, P], dtype=bf16)
            nc.vector.tensor_copy(out=Pb[:], in_=Pf[:])

            ones_col = csb.tile([P, 1], dtype=bf16)
            nc.vector.memset(ones_col[:], 1.0)
            ones_row = csb.tile([1, P], dtype=bf16)
            nc.vector.memset(ones_row[:], 1.0)

            # emission stream chunks (all live; DMA runs ahead of the scan)
            et = []
            for ch in range(NCHB):
                t_ = es.tile([P, G * F], dtype=bf16, tag="e")
                nc.sync.dma_start(out=t_[:], in_=e_d[:, ch * G * F : (ch + 1) * G * F])
                et.append(t_)

            q = []
            for h in range(H):
                q0 = csb.tile([P, FH], dtype=bf16, tag=f"q0_{h}")
                nc.vector.memset(q0[:], 1.0)
                q.append(q0)

            for step in range(1, S + 1):
                ch, off = divmod(step - 1, G)
                for h in range(H):
                    ps = pmm.tile([P, FH], dtype=f32, tag=f"mm{h}")
                    nc.tensor.matmul(
                        out=ps[:], lhsT=Pb[:], rhs=q[h][:], start=True, stop=True
                    )
                    qn = qs.tile([P, FH], dtype=bf16, tag=f"q{h}")
                    c0 = off * F + h * FH
                    nc.vector.tensor_mul(out=qn[:], in0=ps[:], in1=et[ch][:, c0 : c0 + FH])
                    q[h] = qn
                if step in renorms:
                    ri = renorms[step]
                    for h in range(H):
                        cs = pcs.tile([1, FH], dtype=f32, tag="cs")
                        nc.tensor.matmul(
                            out=cs[:], lhsT=ones_col[:, :1], rhs=q[h][:],
                            start=True, stop=True,
                        )
                        lcs = rs.tile([1, FH], dtype=f32, tag="lcs")
                        nc.scalar.activation(out=lcs[:], in_=cs[:], func=LN)
                        nc.sync.dma_start(
                            out=lcs_d[ri : ri + 1, h * FH : (h + 1) * FH],
                            in_=lcs[:1, :],
                        )
                        rcs = rs.tile([1, FH], dtype=bf16, tag="rcs")
                        with nc.allow_low_precision(
                            reason="renorm scale; its rounding lands in the "
                            "measured final mass"
                        ):
                            nc.vector.reciprocal(out=rcs[:], in_=cs[:])
                        bc = pbc.tile([P, FH], dtype=f32, tag="bc")
                        nc.tensor.matmul(
                            out=bc[:], lhsT=ones_row[:1, :], rhs=rcs[:1, :],
                            start=True, stop=True,
                        )
                        qn2 = qs.tile([P, FH], dtype=bf16, tag=f"q{h}")
                        nc.vector.tensor_mul(out=qn2[:], in0=q[h][:], in1=bc[:])
                        q[h] = qn2

            for h in range(H):
                nc.sync.dma_start(out=qf_d[:, h * FH : (h + 1) * FH], in_=q[h][:])

    if not nc.is_finalized():
        nc.finalize()
    return nc


def _get_nc(which):
    if which not in _CACHE:
        _CACHE[which] = _build_nc_a() if which == "a" else _build_nc_b()
    return _CACHE[which]


def _run(x, start_w, start_b, cluster_trans_w, emb_cluster_w, cluster_vocab_w,
         trace=False):
    from concourse.bass_utils import run_bass_kernel_spmd

    x = np.asarray(x).astype(np.int64)
    sw = np.asarray(start_w, np.float32).reshape(K)
    sb = np.asarray(start_b, np.float32).reshape(K)
    tr = np.ascontiguousarray(
        np.asarray(cluster_trans_w, np.float32)[:, 0].reshape(K, K)
    )
    emb = np.asarray(emb_cluster_w, np.float32)
    voc = np.asarray(cluster_vocab_w, np.float32)

    # ---------------- launch A: emission table shards ----------------
    vocT = np.zeros((P, VPAD), ml_dtypes.bfloat16)
    vocT[:, :V] = voc.T.astype(ml_dtypes.bfloat16)
    embT = np.ascontiguousarray(emb.T).astype(ml_dtypes.bfloat16)
    nca = _get_nc("a")
    in_a = [
        {"vocT": np.ascontiguousarray(vocT[:, c * VSH : (c + 1) * VSH]), "embT": embT}
        for c in range(8)
    ]
    ra = run_bass_kernel_spmd(nca, in_a, list(range(8)), trace=trace)
    exec_a = ra.exec_time_ns
    res_a = ra.results

    TAB = np.concatenate(
        [res_a[c]["tab"].astype(np.float32) for c in range(8)], axis=1
    )  # (K, VPAD)
    s = np.sum([res_a[c]["sums"][:, 0].astype(np.float64) for c in range(8)], axis=0)

    # ---------------- host: normalize, gather, build lane streams ----------------
    En = TAB[:, :V] / s[:, None].astype(np.float32)        # (K, V) true p(v|k)
    Gall = En[:, x.reshape(-1)]                            # (K, N*T)
    lnkap = -float(np.mean(np.log(Gall.mean(axis=0, dtype=np.float64))))
    Gall *= np.float32(np.exp(lnkap))                      # centered emissions
    Gall = Gall.reshape(K, N, T)

    # transition (f64) for the exact chunk-0 warmup column
    trd = tr.astype(np.float64)
    Pd = np.exp(trd - trd.max(1, keepdims=True))
    Pd /= Pd.sum(1, keepdims=True)
    A = Pd.T
    w = np.ones(K)
    for st_ in range(1, W):
        w = A @ w
        if st_ in RENS:
            w = w / w.sum()
    p0 = np.exp((sw + sb).astype(np.float64))
    p0col = (p0 / (A @ w)).astype(np.float32)

    # per-(step, chunk) real-token index; -1 = ones, -2 = p0 column
    tmap = np.empty((S, C), np.int64)
    for si in range(S):
        step = si + 1
        for c in range(C):
            if c == 0:
                tmap[si, 0] = -1 if step < W else (-2 if step == W else step - W - 1)
            else:
                tmap[si, c] = c * L - W + step - 1
    ones_mask = tmap == -1
    p0_mask = tmap == -2
    tclip = np.clip(tmap, 0, T - 1)

    e_maps = []
    for cc in range(8):
        st = np.empty((S, NSEQ, C, K), np.float32)
        for nl in range(NSEQ):
            n = cc * NSEQ + nl
            v = st[:, nl]                                   # view (S,C,K)
            v[...] = Gall[:, n, tclip].transpose(1, 2, 0)
            v[ones_mask] = 1.0
            v[p0_mask] = p0col
        # column index = si*F + nl*C + c  -> (S, NSEQ, C) ordering is exactly that
        e_maps.append(
            {
                "e": np.ascontiguousarray(
                    st.reshape(S * F, K).T.astype(ml_dtypes.bfloat16)
                ),
                "tr": tr,
            }
        )

    # ---------------- launch B: chunked scan ----------------
    ncb = _get_nc("b")
    rb = run_bass_kernel_spmd(ncb, e_maps, list(range(8)), trace=trace)
    exec_b = rb.exec_time_ns
    res_b = rb.results

    # ---------------- host: stitch ----------------
    losses = np.empty(N, np.float64)
    for cc in range(8):
        qf = res_b[cc]["qf"].astype(np.float64)            # (K, F)
        lcs = res_b[cc]["lcs"].astype(np.float64)          # (NREN, F)
        contrib = lcs[1:].sum(axis=0) + np.log(qf.sum(axis=0))  # (F,)
        contrib = contrib.reshape(NSEQ, C)
        contrib[:, 0] += lcs[0].reshape(NSEQ, C)[:, 0]     # chunk-0 boundary mass
        for nl in range(NSEQ):
            n = cc * NSEQ + nl
            losses[n] = -(contrib[nl].sum() - T * lnkap)
    return np.float32(losses.mean()), (exec_a, exec_b)


def kernel(x, start_w, start_b, cluster_trans_w, emb_cluster_w, cluster_vocab_w):
    loss, _ = _run(x, start_w, start_b, cluster_trans_w, emb_cluster_w,
                   cluster_vocab_w)
    return loss
